# revision 42
# baseline (speedup 1.0000x reference)
"""DMPNN layer kernel for 8 Trainium2 NeuronCores.

Sharding: data-parallel over destination nodes j (dim 2 of edge_attr/adj,
dim 1 of the output). Each core gets a 64-column j-slice of edge_attr/adj,
the full h (needed because messages sum over all source nodes i), and the
small weights replicated. The batch-global mask (adj.sum(0) > 0) only needs
the core's own j-slice of adj over the full batch, so no collective at all.

Math per core (j in its 64-column slice, source nodes i = 4p + q):
  mask[i,j]   = max_b adj[b,i,j]                    (adj is 0/1)
  deg[j]      = sum_i mask[i,j]
  mh[b,j,f]   = sum_i mask[i,j] h[b,i,f]
  me[b,j,e]   = sum_i mask[i,j] edge[b,i,j,e]
  msg[b,j,o]  = sum_f Wh[o,f] mh[b,j,f] + deg[j] wb[o] + sum_e We[o,e] me[b,j,e]
  out[b,j,o]  = sum_f U[o,f] (h[b,j,f] + msg[b,j,f]) + ub[o]

Design (measured on this part: PE capped at ~1.2 GHz with ~165ns/instr
overhead, DVE ~1 elem/cyc/lane fp32 with ~0.3-0.4us/op overhead and 6x
penalties for strided reads OR writes, dma_start occupies its issuing
engine ~0.7us, HWDGE ~330 GB/s for 1MiB+ transfers, SWDGE cast ~146 GB/s):
 - sync queue: half of adj, then the 8x1MiB edge stream (fully contiguous
   8 KiB/partition descriptors); scalar queue: other adj half, h, weights;
   gpsimd/SWDGE: small structural partition-move DMAs + output stores.
 - mask via a 7-op pairwise max tree on contiguous slices.
 - bf16 on every hot matmul path (fp32 matmul = 2x LOW_HIGH passes).
 - per batch: ONE DVE mask-multiply (fp32 edge x broadcast mask -> bf16)
   and ONE q-fold add (outer-strided views, contiguous 512-elem runs),
   then 1-2 accumulating i-reduce matmuls whose e-major-strided rhs AP
   makes psum_e land directly as me^T-flat (cheapest home for the
   (j,e)->(e,j) remap); fold depth alternates per batch to balance PE/DVE.
 - mh matmuls carry a 65th all-ones lhsT column so psum row 64 accumulates
   deg for free; one ACT copy drops [mhT; deg] into the stacked rhs and an
   8-descriptor SWDGE DMA adds me^T rows.
 - ONE matmul for the whole message: lhsT [Wh^T; wb; We^T] (73 rows) x
   rhs [mhT; deg; me^T]; ONE matmul for the output: [U^T; ub] x [XT; ones].
 - batch-0 work is emitted before the weight-transpose prep so no engine
   FIFO blocks the first edge tiles; s1/s2/s3 run with 1/2-batch lags.
"""

import numpy as np


def _ensure_path():
    try:
        import concourse.bass  # noqa: F401
    except ImportError:
        import sys

        for p in ("/opt/trn_rl_repo", "/root/.axon_site/_ro/trn_rl_repo"):
            if p not in sys.path:
                sys.path.insert(0, p)


B, N, H, E = 8, 512, 64, 8
NCORES = 8
JB = N // NCORES  # 64 destination columns per core
CH = N // 128  # 4 source-node sub-chunks (i = 4p + q)


_CACHE = {}


def _build_program():
    _ensure_path()
    import concourse.bacc as bacc
    import concourse.mybir as mybir
    import concourse.tile as tile

    dt = mybir.dt
    f32 = dt.float32
    bf16 = dt.bfloat16
    i32 = dt.int32
    Alu = mybir.AluOpType
    Axis = mybir.AxisListType

    nc = bacc.Bacc("TRN2", debug=False, num_devices=NCORES)

    edge = nc.dram_tensor("edge", [B, N, JB, E], f32, kind="ExternalInput").ap()
    adjs = nc.dram_tensor("adjs", [B, N, JB], i32, kind="ExternalInput").ap()
    h = nc.dram_tensor("h", [B, N, H], f32, kind="ExternalInput").ap()
    hs = nc.dram_tensor("hs", [B, JB, H], f32, kind="ExternalInput").ap()
    Ww = nc.dram_tensor("Ww", [H, H + E], f32, kind="ExternalInput").ap()
    Wb = nc.dram_tensor("Wb", [1, H], f32, kind="ExternalInput").ap()
    Uw = nc.dram_tensor("Uw", [H, H], f32, kind="ExternalInput").ap()
    Ub = nc.dram_tensor("Ub", [1, H], f32, kind="ExternalInput").ap()
    out = nc.dram_tensor("out", [B, H, JB], f32, kind="ExternalOutput").ap()

    ident_d = nc.inline_tensor(np.eye(64, dtype=np.float32), "ident")

    KM = H + 1 + E  # 73 contraction rows of the fused message matmul
    KU = H + 1  # 65 contraction rows of the fused output matmul
    HD = H + 1  # h chunk width incl. the ones column (deg rides along)

    with tile.TileContext(nc) as tc:
        with (
            tc.tile_pool(name="const", bufs=1) as cpool,
            tc.tile_pool(name="masked", bufs=3) as mpool,
            tc.tile_pool(name="acc", bufs=4) as apool,
            tc.tile_pool(name="small", bufs=4) as spool,
            tc.tile_pool(name="pe", bufs=2, space="PSUM") as ppool_e,
            tc.tile_pool(name="pmh", bufs=2, space="PSUM") as ppool_mh,
            tc.tile_pool(name="pmsg", bufs=2, space="PSUM") as ppool_msg,
            tc.tile_pool(name="pout", bufs=2, space="PSUM") as ppool_out,
        ):
            # ---- adj first on sync: it gates the mask -> everything ----
            adj_sb = cpool.tile([128, B * CH * JB], i32)
            adj_o = adj_sb.rearrange("p (b qj) -> p b qj", b=B)
            adj_i = adjs.rearrange("b (p q) j -> p b (q j)", q=CH)
            # halves on both HWDGE queues in parallel: mask tree starts ~1us
            # sooner and the edge stream follows immediately on sync
            nc.sync.dma_start(out=adj_o[:, 0:4], in_=adj_i[:, 0:4])
            nc.scalar.dma_start(out=adj_o[:, 4:8], in_=adj_i[:, 4:8])

            # ---- edge: the 8 MiB stream on sync; contiguous 8 KiB/partition
            edge_t = [
                cpool.tile([128, CH * JB * E], f32, name=f"edge{b}") for b in range(B)
            ]
            EH = CH // 2 * JB * E  # 1024: half a batch (q0,q1)
            for b in range(B):
                src = edge[b].rearrange("(p q) j e -> p (q j e)", q=CH)
                if b == B - 1:
                    # last batch in halves: its reduce overlaps the arrival
                    # and multiply of the second half (pure tail latency)
                    nc.sync.dma_start(out=edge_t[b][:, 0:EH], in_=src[:, 0:EH])
                    nc.sync.dma_start(out=edge_t[b][:, EH:], in_=src[:, EH:])
                else:
                    nc.sync.dma_start(out=edge_t[b][:, :], in_=src)

            # ---- scalar queue: tiny weights FIRST (they gate the PE
            # transposes at the FIFO head), then h, then hs ----
            ident = cpool.tile([64, 64], f32)
            nc.scalar.dma_start(out=ident[:, :], in_=ident_d.ap()[:, :])
            Ww_sb = cpool.tile([H, H + E], f32)
            nc.scalar.dma_start(out=Ww_sb[:, :], in_=Ww[:, :])
            Uw_sb = cpool.tile([H, H], f32)
            nc.scalar.dma_start(out=Uw_sb[:, :], in_=Uw[:, :])
            wb_sb = cpool.tile([1, H], f32)
            nc.scalar.dma_start(out=wb_sb[:, :], in_=Wb[:, :])
            ub_sb = cpool.tile([1, H], f32)
            nc.scalar.dma_start(out=ub_sb[:, :], in_=Ub[:, :])
            # h via SWDGE (plain copy, full rate): keeps the scalar queue
            # free for the small latency-critical per-batch transfers
            h_f32 = cpool.tile([128, B * CH * H], f32)
            nc.gpsimd.dma_start(
                out=h_f32.rearrange("p (b qf) -> p b qf", b=B),
                in_=h.rearrange("b (p q) f -> p b (q f)", q=CH),
            )
            hs_all = cpool.tile([JB, B * H], f32)
            nc.gpsimd.dma_start(
                out=hs_all.rearrange("j (b f) -> j b f", b=B),
                in_=hs.rearrange("b j f -> j b f"),
            )
            # ---- constants ----
            ones_bf = cpool.tile([128, 1], bf16)
            nc.vector.memset(ones_bf[:, :], 1.0)

            # ---- mask: pairwise max tree (contiguous reads beat the
            # strided tensor_reduce; first ops start on the adj half) ----
            adj_v = adj_sb.rearrange("p (b qj) -> p b qj", b=B)
            mt0 = cpool.tile([128, CH * JB], i32, name="mt0")
            mt1 = cpool.tile([128, CH * JB], i32, name="mt1")
            mt2 = cpool.tile([128, CH * JB], i32, name="mt2")
            mt3 = cpool.tile([128, CH * JB], i32, name="mt3")
            nc.vector.tensor_tensor(mt0[:, :], adj_v[:, 0], adj_v[:, 1], Alu.max)
            nc.vector.tensor_tensor(mt1[:, :], adj_v[:, 2], adj_v[:, 3], Alu.max)
            nc.vector.tensor_tensor(mt2[:, :], adj_v[:, 4], adj_v[:, 5], Alu.max)
            nc.vector.tensor_tensor(mt3[:, :], adj_v[:, 6], adj_v[:, 7], Alu.max)
            nc.vector.tensor_tensor(mt0[:, :], mt0[:, :], mt1[:, :], Alu.max)
            nc.vector.tensor_tensor(mt2[:, :], mt2[:, :], mt3[:, :], Alu.max)
            mask_f = cpool.tile([128, CH * JB], f32)
            nc.vector.tensor_tensor(mask_f[:, :], mt0[:, :], mt2[:, :], Alu.max)
            mask_bf = cpool.tile([128, CH * JB], bf16)
            nc.vector.tensor_copy(mask_bf[:, :], mask_f[:, :])

            # ---- h cast to bf16 with an interleaved ones column ----
            # h_bf2[p, g*65 : g*65+64] = h chunk g (bf16), [.., g*65+64] = 1.0
            h_bf2 = cpool.tile([128, B * CH * HD], bf16)
            hb_v = h_bf2.rearrange("p (g x) -> p g x", x=HD)
            nc.scalar.copy(
                out=hb_v[:, :, 0:H],
                in_=h_f32.rearrange("p (g f) -> p g f", f=H),
            )
            nc.vector.memset(hb_v[:, :, H : H + 1], 1.0)

            # ---- weight transposes + bf16 casts ----
            Wh2 = cpool.tile([H, H], bf16)
            WeM8 = cpool.tile([E, H], bf16)
            U2 = cpool.tile([H, H], bf16)
            wb_bf = cpool.tile([1, H], bf16)
            ub_bf = cpool.tile([1, H], bf16)
            nc.scalar.copy(wb_bf[:, :], wb_sb[:, :])
            nc.scalar.copy(ub_bf[:, :], ub_sb[:, :])

            # hs transposed: hsT_all[f, (b j)]  (tile defs; instructions
            # are emitted after s1(0) so batch-0 work leads every FIFO)
            hsT_all = cpool.tile([H, B * JB], f32)
            WWb = cpool.tile([KM, H], bf16)  # [Wh^T; wb; We^T]
            UUb = cpool.tile([KU, H], bf16)  # [U^T; ub]

            def emit_weight_prep():
                pwh = ppool_out.tile([H, H], f32, tag="o", name="pwh")
                nc.tensor.transpose(pwh[:, :], Ww_sb[:, 0:H], ident[0:H, 0:H])
                nc.scalar.copy(Wh2[:, :], pwh[:, :])

                pwe = ppool_out.tile([E, H], f32, tag="o", name="pwe")
                nc.tensor.transpose(pwe[:, :], Ww_sb[:, H : H + E], ident[0:H, 0:H])
                nc.scalar.copy(WeM8[:, :], pwe[:, :])

                puw = ppool_out.tile([H, H], f32, tag="o", name="puw")
                nc.tensor.transpose(puw[:, :], Uw_sb[:, :], ident[0:H, 0:H])
                nc.scalar.copy(U2[:, :], puw[:, :])

                for b in range(B):
                    pht = ppool_msg.tile([H, JB], f32, tag="m", name="pht")
                    nc.tensor.transpose(
                        pht[:, :], hs_all[:, b * H : (b + 1) * H], ident[0:JB, 0:JB]
                    )
                    nc.scalar.copy(hsT_all[:, b * JB : (b + 1) * JB], pht[:, :])

                # stacked stationary operands (partition moves via SWDGE)
                nc.gpsimd.dma_start(out=WWb[0:H, :], in_=Wh2[:, :])
                nc.gpsimd.dma_start(out=WWb[H : H + 1, :], in_=wb_bf[:, :])
                nc.gpsimd.dma_start(out=WWb[H + 1 : KM, :], in_=WeM8[:, :])
                nc.gpsimd.dma_start(out=UUb[0:H, :], in_=U2[:, :])
                nc.gpsimd.dma_start(out=UUb[H : H + 1, :], in_=ub_bf[:, :])

            # stacked rhs buffers: mhTd[b] = [mhT; deg; me^T]
            mhTd = [cpool.tile([KM, JB], bf16, name=f"mhTd{b}") for b in range(B)]

            # XT buffers: [msgT + hsT; ones]
            NXT = 4
            XT = [cpool.tile([KU, JB], bf16, name=f"XT{b}") for b in range(NXT)]
            for i in range(NXT):
                nc.vector.memset(XT[i][H : H + 1, :], 1.0)

            # broadcast view of the f32 mask over the e axis (stride-0)
            mask_q = mask_f.rearrange("p (q j) -> p q j", q=CH)

            # ---------------- per-batch software pipeline ----------------
            st = [dict() for _ in range(B)]

            def s1(b):
                masked = mpool.tile([128, CH * JB * E], bf16, name="masked")
                mk_v = masked.rearrange("p (q j e) -> p q j e", q=CH, j=JB)
                eg_v = edge_t[b].rearrange("p (q j e) -> p q j e", q=CH, j=JB)
                psum_e = ppool_e.tile([1, JB * E], f32, name="psum_e")
                d = st[b]
                if b == B - 1:
                    # last batch: halves, so fold/matmul overlap the second
                    # half's DMA + multiply instead of serializing after it
                    HJE2 = JB * E
                    acc2l = apool.tile([128, 2 * HJE2], bf16, name="acc2l")
                    for half in range(2):
                        q0 = 2 * half
                        nc.vector.tensor_tensor(
                            out=mk_v[:, q0 : q0 + 2],
                            in0=eg_v[:, q0 : q0 + 2],
                            in1=mask_q[:, q0 : q0 + 2].broadcast_to(
                                [128, 2, JB, E]
                            ),
                            op=Alu.mult,
                        )
                        nc.vector.tensor_tensor(
                            out=acc2l[:, half * HJE2 : (half + 1) * HJE2],
                            in0=masked[:, q0 * HJE2 // 2 * 2 : (q0 + 1) * HJE2],
                            in1=masked[:, (q0 + 1) * HJE2 : (q0 + 2) * HJE2],
                            op=Alu.add,
                        )
                        nc.tensor.matmul(
                            psum_e[:, :],
                            lhsT=ones_bf[:, :],
                            rhs=acc2l[
                                :, half * HJE2 : (half + 1) * HJE2
                            ].rearrange("p (j e) -> p e j", e=E),
                            start=(half == 0),
                            stop=(half == 1),
                        )
                    d["psum_e"] = psum_e
                    psum_mhT = ppool_mh.tile([HD, JB], f32, name="psum_mhT")
                    for c in range(CH):
                        g = b * CH + c
                        nc.tensor.matmul(
                            psum_mhT[:, :],
                            lhsT=h_bf2[:, g * HD : (g + 1) * HD],
                            rhs=mask_bf[:, c * JB : (c + 1) * JB],
                            start=(c == 0),
                            stop=(c == CH - 1),
                        )
                    d["psum_mhT"] = psum_mhT
                    return
                # one mask-multiply op for the whole batch (DVE per-op
                # overhead is ~0.4us, so fewer/bigger ops win)
                nc.vector.tensor_tensor(
                    out=mk_v[:, :],
                    in0=eg_v[:, :],
                    in1=mask_q[:, :].broadcast_to([128, CH, JB, E]),
                    op=Alu.mult,
                )
                # one q-fold op: (q0+q1 | q2+q3) -> acc2 halves; the views
                # stride only at the outer dim (contiguous 512-elem runs),
                # which DVE handles at full rate (inner strides do not).
                HJE = JB * E
                mgq = masked.rearrange("p (g q x) -> p g q x", g=2, x=HJE)
                acc2 = apool.tile([128, 2 * HJE], bf16, name="acc2")
                nc.vector.tensor_tensor(
                    out=acc2.rearrange("p (g x) -> p g x", g=2),
                    in0=mgq[:, :, 0],
                    in1=mgq[:, :, 1],
                    op=Alu.add,
                )
                # alternate fold depth per batch to balance PE vs DVE:
                # even b -> two accumulating matmuls; odd b -> one extra
                # DVE add and a single matmul.
                if b % 4 == 0:
                    for half in range(2):
                        nc.tensor.matmul(
                            psum_e[:, :],
                            lhsT=ones_bf[:, :],
                            rhs=acc2[:, half * HJE : (half + 1) * HJE].rearrange(
                                "p (j e) -> p e j", e=E
                            ),
                            start=(half == 0),
                            stop=(half == 1),
                        )
                else:
                    asum = apool.tile([128, HJE], bf16, name="asum")
                    nc.vector.tensor_tensor(
                        out=asum[:, :],
                        in0=acc2[:, 0:HJE],
                        in1=acc2[:, HJE:],
                        op=Alu.add,
                    )
                    nc.tensor.matmul(
                        psum_e[:, :],
                        lhsT=ones_bf[:, :],
                        rhs=asum.rearrange("p (j e) -> p e j", e=E),
                        start=True,
                        stop=True,
                    )
                d["psum_e"] = psum_e

                # mh (+deg via the 65th ones column) - needs only mask + h;
                # emitted after the edge matmuls so they are never blocked
                psum_mhT = ppool_mh.tile([HD, JB], f32, name="psum_mhT")
                for c in range(CH):
                    g = b * CH + c
                    nc.tensor.matmul(
                        psum_mhT[:, :],
                        lhsT=h_bf2[:, g * HD : (g + 1) * HD],
                        rhs=mask_bf[:, c * JB : (c + 1) * JB],
                        start=(c == 0),
                        stop=(c == CH - 1),
                    )
                d["psum_mhT"] = psum_mhT

            def s2(b):
                # psum_e is already e-major: one contiguous PSUM->SBUF copy,
                # then an 8-descriptor partition-move into the stacked rhs.
                d = st[b]
                nc.scalar.copy(mhTd[b][0:HD, :], d["psum_mhT"][:, :])
                me_sb = spool.tile([1, JB * E], bf16, name="me_sb")
                nc.scalar.copy(out=me_sb[:, :], in_=d["psum_e"][:, :])
                nc.scalar.dma_start(
                    out=mhTd[b][H + 1 : KM, :],
                    in_=me_sb.rearrange("p (e j) -> p e j", e=E),
                )

            def s3(b):
                psum_msgT = ppool_msg.tile([H, JB], f32, tag="m", name="psum_msgT")
                nc.tensor.matmul(
                    psum_msgT[:, :], lhsT=WWb[:, :], rhs=mhTd[b][:, :],
                    start=True, stop=True,
                )
                xt = XT[b % NXT]
                nc.vector.tensor_tensor(
                    out=xt[0:H, :],
                    in0=psum_msgT[:, :],
                    in1=hsT_all[:, b * JB : (b + 1) * JB],
                    op=Alu.add,
                )
                psum_outT = ppool_out.tile([H, JB], f32, tag="o", name="psum_outT")
                nc.tensor.matmul(
                    psum_outT[:, :], lhsT=UUb[:, :], rhs=xt[:, :],
                    start=True, stop=True,
                )
                out_sb = spool.tile([H, JB], f32, name="out_sb")
                nc.scalar.copy(out_sb[:, :], psum_outT[:, :])
                # HWDGE stores: the SWDGE queue serializes tiny DMAs at
                # ~2.6us each, which would gate the whole back half
                nc.scalar.dma_start(out=out[b], in_=out_sb[:, :])

            LAG2, LAG3 = 1, 3
            s1(0)
            emit_weight_prep()
            for t in range(1, B + LAG3):
                if t < B:
                    s1(t)
                if LAG2 <= t < B + LAG2:
                    s2(t - LAG2)
                if LAG3 <= t:
                    s3(t - LAG3)

    nc.compile()
    return nc


def _get_program():
    if "nc" not in _CACHE:
        _CACHE["nc"] = _build_program()
    return _CACHE["nc"]


def _make_in_maps(h, edge_attr, adj, W_w, W_b, U_w, U_b):
    h = np.ascontiguousarray(np.asarray(h, dtype=np.float32))
    edge_attr = np.asarray(edge_attr, dtype=np.float32)
    adj = np.asarray(adj, dtype=np.int32)
    W_w = np.ascontiguousarray(np.asarray(W_w, dtype=np.float32))
    W_b = np.ascontiguousarray(np.asarray(W_b, dtype=np.float32)).reshape(1, H)
    U_w = np.ascontiguousarray(np.asarray(U_w, dtype=np.float32))
    U_b = np.ascontiguousarray(np.asarray(U_b, dtype=np.float32)).reshape(1, H)

    in_maps = []
    for c in range(NCORES):
        j0 = c * JB
        in_maps.append(
            {
                "edge": np.ascontiguousarray(edge_attr[:, :, j0 : j0 + JB, :]),
                "adjs": np.ascontiguousarray(adj[:, :, j0 : j0 + JB]),
                "h": h,
                "hs": np.ascontiguousarray(h[:, j0 : j0 + JB, :]),
                "Ww": W_w,
                "Wb": W_b,
                "Uw": U_w,
                "Ub": U_b,
            }
        )
    return in_maps


def _install_ntff_hook():
    """The agent image lacks antenv.axon_hooks; synthesize it so trace=True
    can reach the libaxon NTFF profiling entry points."""
    import sys
    import types

    try:
        from antenv.axon_hooks import get_axon_ntff_profile_hook  # noqa: F401

        return
    except ImportError:
        pass
    import antenv

    mod = types.ModuleType("antenv.axon_hooks")
    _h = [None]
    mod.set_axon_ntff_profile_hook = lambda hook: _h.__setitem__(0, hook)
    mod.get_axon_ntff_profile_hook = lambda: _h[0]
    sys.modules["antenv.axon_hooks"] = mod
    antenv.axon_hooks = mod
    try:
        from trn_agent_boot.trn_boot import _ntff_profile_via_ctypes

        mod.set_axon_ntff_profile_hook(
            _ntff_profile_via_ctypes("/opt/axon/libaxon_pjrt.so")
        )
    except Exception:
        pass
    # avoid the bucket upload (no bucket in this container)
    import concourse.bass_utils as bu

    bu.upload_artifacts = lambda tmpdir: str(tmpdir)


def run(h, edge_attr, adj, W_w, W_b, U_w, U_b, trace=False, trace_cores=None):
    """Run the kernel; returns (output, BassKernelResults)."""
    _ensure_path()
    if trace:
        _install_ntff_hook()
    from concourse.bass_utils import run_bass_kernel_spmd

    nc = _get_program()
    in_maps = _make_in_maps(h, edge_attr, adj, W_w, W_b, U_w, U_b)
    kw = {}
    if trace:
        kw = {"trace": True, "trace_cores": trace_cores or [0]}
    res = run_bass_kernel_spmd(nc, in_maps, list(range(NCORES)), **kw)
    outs = [res.results[c]["out"].transpose(0, 2, 1) for c in range(NCORES)]
    full = np.concatenate(outs, axis=1)  # [B, N, H]
    return full, res


def kernel(h, edge_attr, adj, W_w, W_b, U_w, U_b):
    full, _ = run(h, edge_attr, adj, W_w, W_b, U_w, U_b)
    return full
